# revision 13
# baseline (speedup 1.0000x reference)
"""ECE loss kernel for Trainium2 (Bass/Tile), data-parallel over 8 NeuronCores.

Math (per sample row of logits[N, C]):
  conf = max softmax(x) = exp(max(x)) / sum(exp(x))
  acc  = (argmax(x) == label)
  ece  = sum_b |conf_sum[b] - acc_sum[b]| / N   over 16 ceil-bins of conf

Device work per core (125k rows as [125 partitions x 1000 samples]):
  - DMA 22 tiles of x ([125, k, 100] f32, ~2.5MB each) on 3 DMA queues
  - ACT: E = exp(x) in place (monotone, so row-max of E works for conf/acc)
  - PE : per-sample class sums via PSUM accumulation: 10 identity-matmuls
         per tile, each over a 10-class slice (fp32r, 1 cyc/row), leaving
         psum[p, k, 10] = partial sums; DVE finishes with a width-10 reduce.
         This moves the big class-sum reduction off the Vector engine.
  - DVE: row-max reduce of E (the one remaining full pass), then per-chunk
         tail: sinv = recip(s), acc = (exp(g) == max E), conf = max*sinv,
         v = 2*acc + conf.
  - Binning via per-threshold accumulated sums, split across engines:
      ACT: R_b = sum relu(conf - C_b)   (bin mass via complement)
           T   = sum conf
           A_b = sum sign(v - (2+C_b))  (acc counts via sign-parity)
           n1  = sum acc
      DVE: NN_b = sum (conf <= C_b)     (exact counts)
    done twice (chunks of 500 samples/partition) so the second half
    overlaps the first half's tail instead of serializing at the end.
  Host recovers per-bin sums:
      S_b = T - R_b - C_b*(n - NN_b)         (cum conf mass up to C_b)
      K_b = (n - A_b)/2 ; cumacc_b = K_b - n0
      conf_sum/acc_sum by first differences; ece = sum |.| / N
C_b is the exact f32 boundary: the largest f32 y with f32(15*y) <= b+1, so
binning matches the reference's ceil(conf*15) bit-for-bit.
"""

import os

import numpy as np

import concourse.bass as bass
import concourse.mybir as mybir
import concourse.tile as tile
from concourse.bass_utils import run_bass_kernel_spmd

F32 = mybir.dt.float32
F32R = mybir.dt.float32r
ALU = mybir.AluOpType
AX = mybir.AxisListType
ACTF = mybir.ActivationFunctionType

N = 1_000_000
C = 100
NCORES = 8
ROWS = N // NCORES          # 125000 rows per core
P = 125                     # SBUF partitions used
SPP = ROWS // P             # 1000 samples per partition
TILE_K = 50                 # samples per partition per tile
NBINS = 16                  # 15 real bins + always-empty tail bin
NGRP = 10                   # classes per matmul group (C / NGRP matmuls/tile)
NCHUNK = 2                  # binning chunks (SPP / NCHUNK samples each)

LAST_RESULTS = None         # stashed BassKernelResults for test harness


def _bin_thresholds():
    """C_b = largest f32 y such that f32(15*y) <= b+1, for b = 0..14."""
    thr = []
    for b in range(15):
        tgt = np.float32(b + 1)

        def f(v):
            return np.float32(np.float32(15.0) * v)

        y = np.float32((b + 1) / 15.0)
        if f(y) <= tgt:
            while True:
                y2 = np.nextafter(y, np.float32(np.inf))
                if f(y2) <= tgt:
                    y = y2
                else:
                    break
        else:
            while f(y) > tgt:
                y = np.nextafter(y, np.float32(-np.inf))
        thr.append(np.float32(y))
    return thr


THR = _bin_thresholds()


def _build():
    nc = bass.Bass(trn_type="TRN2")
    x = nc.dram_tensor("x", [P, SPP * C], F32, kind="ExternalInput")
    g = nc.dram_tensor("g", [P, SPP], F32, kind="ExternalInput")
    eye = nc.dram_tensor("eye", [P, P], F32, kind="ExternalInput")
    thr = nc.dram_tensor("thr", [P, 32], F32, kind="ExternalInput")
    r_out = nc.dram_tensor("r", [P, NCHUNK * NBINS], F32, kind="ExternalOutput")
    nn_out = nc.dram_tensor("nn", [P, NCHUNK * NBINS], F32, kind="ExternalOutput")
    a_out = nc.dram_tensor("a", [P, NCHUNK * NBINS], F32, kind="ExternalOutput")

    X = x[:, :].rearrange("p (k c) -> p k c", c=C)  # [125, 1000, 100]

    # small leading tiles so the first transfers land quickly and the
    # compute pipeline starts sooner; chunk boundary (sample 500) falls
    # exactly after tile index 11
    sizes = [12, 13, 25] + [TILE_K] * 19
    assert sum(sizes) == SPP
    CHUNK = SPP // NCHUNK
    BUFS = 5

    with tile.TileContext(nc) as tc:
        with (
            tc.tile_pool(name="xin", bufs=BUFS) as xin,
            tc.tile_pool(name="epool", bufs=4) as epool,
            tc.tile_pool(name="persist", bufs=1) as persist,
            tc.psum_pool(name="ps", bufs=2) as ps,
        ):
            BF16 = mybir.dt.bfloat16
            em_bf = persist.tile([P, SPP], BF16)
            em_all = persist.tile([P, SPP], F32)
            s_all = persist.tile([P, SPP], F32)
            g_sb = persist.tile([P, SPP], F32)
            eg_bf = persist.tile([P, SPP], BF16)
            v_sb = persist.tile([P, SPP], F32)
            eye_sb = persist.tile([P, P], F32)
            eye_bf = persist.tile([P, P], BF16)
            r_sb = persist.tile([P, NCHUNK * NBINS], F32)
            nn_sb = persist.tile([P, NCHUNK * NBINS], F32)
            a_sb = persist.tile([P, NCHUNK * NBINS], F32)
            act_scr = persist.tile([P, CHUNK], F32)
            dve_scr = persist.tile([P, CHUNK], F32)
            thr_sb = persist.tile([P, 32], F32)
            nc.gpsimd.dma_start(out=g_sb[:, :], in_=g[:, :])
            nc.gpsimd.dma_start(out=eye_sb[:, :], in_=eye[:, :])
            nc.gpsimd.dma_start(out=thr_sb[:, :], in_=thr[:, :])
            nc.scalar.activation(eye_bf[:, :], eye_sb[:, :], ACTF.Copy)

            dma_engines = [nc.sync, nc.scalar, nc.gpsimd]

            def tail_chunk(c):
                cs = slice(c * CHUNK, (c + 1) * CHUNK)
                nc.vector.reciprocal(s_all[:, cs], s_all[:, cs])
                # exp(g) rounded to bf16 exactly like the tile exps, so the
                # accuracy equality-compare matches bit-for-bit when the
                # label hits the argmax
                nc.scalar.activation(eg_bf[:, cs], g_sb[:, cs], ACTF.Exp)
                # em to f32 for the conf arithmetic (exact widening)
                nc.scalar.activation(em_all[:, cs], em_bf[:, cs], ACTF.Copy)
                # acc (in g_sb), conf (in em_all), v = 2*acc + conf
                nc.vector.tensor_tensor(
                    g_sb[:, cs], eg_bf[:, cs], em_bf[:, cs], op=ALU.is_equal
                )
                nc.vector.tensor_tensor(
                    em_all[:, cs], em_all[:, cs], s_all[:, cs], op=ALU.mult
                )
                nc.vector.scalar_tensor_tensor(
                    v_sb[:, cs], g_sb[:, cs], 2.0, em_all[:, cs],
                    op0=ALU.mult, op1=ALU.add,
                )
                conf = em_all[:, cs]
                for b in range(15):
                    cb = float(THR[b])
                    nc.scalar.activation(
                        act_scr[:, :], conf, ACTF.Relu,
                        bias=thr_sb[:, b : b + 1],
                        accum_out=r_sb[:, c * NBINS + b : c * NBINS + b + 1],
                    )
                    nc.vector.tensor_scalar(
                        dve_scr[:, :], conf, cb, None,
                        op0=ALU.is_le, op1=ALU.add,
                        accum_out=nn_sb[:, c * NBINS + b : c * NBINS + b + 1],
                    )
                    nc.scalar.activation(
                        act_scr[:, :], v_sb[:, cs], ACTF.Sign,
                        bias=thr_sb[:, 15 + b : 16 + b],
                        accum_out=a_sb[:, c * NBINS + b : c * NBINS + b + 1],
                    )
                nc.scalar.activation(
                    act_scr[:, :], conf, ACTF.Identity, bias=0.0,
                    accum_out=r_sb[:, c * NBINS + 15 : c * NBINS + 16],
                )
                nc.scalar.activation(
                    act_scr[:, :], g_sb[:, cs], ACTF.Identity, bias=0.0,
                    accum_out=a_sb[:, c * NBINS + 15 : c * NBINS + 16],
                )

            off = 0
            for t, k in enumerate(sizes):
                sl = slice(off, off + k)
                off += k
                xt = xin.tile([P, TILE_K, C], F32, tag="xt")
                dma_engines[t % len(dma_engines)].dma_start(
                    out=xt[:, :k, :], in_=X[:, sl, :]
                )
                # E = exp(x) in bf16: row-max is over E (exp is monotone) and
                # the accuracy compare uses the same rounded spline output;
                # bf16 E feeds the Tensor engine at 1 cycle/row
                et = epool.tile([P, TILE_K, C], BF16, tag="et")
                nc.scalar.activation(et[:, :k, :], xt[:, :k, :], ACTF.Exp)
                # per-sample class sums: 10-class slices accumulated in PSUM
                pt = ps.tile([P, TILE_K, NGRP], F32, tag="ps")
                for gi in range(C // NGRP):
                    nc.tensor.matmul(
                        pt[:, :k, :],
                        eye_bf[:, :],
                        et[:, :k, gi * NGRP : (gi + 1) * NGRP],
                        start=(gi == 0),
                        stop=(gi == C // NGRP - 1),
                    )
                nc.vector.reduce_sum(out=s_all[:, sl], in_=pt[:, :k, :], axis=AX.X)
                nc.vector.reduce_max(out=em_bf[:, sl], in_=et[:, :k, :], axis=AX.X)
                if off == CHUNK:
                    tail_chunk(0)
            tail_chunk(1)

            nc.sync.dma_start(out=r_out[:, :], in_=r_sb[:, :])
            nc.sync.dma_start(out=nn_out[:, :], in_=nn_sb[:, :])
            nc.sync.dma_start(out=a_out[:, :], in_=a_sb[:, :])

    import bass_rust as _br

    # Instructions carry at most 2 sync commands (waits + completion update),
    # so any instruction the Tile scheduler gave >1 wait has its extra waits
    # peeled onto same-engine drains inserted just before it.
    for bb in nc.m.functions[0].blocks:
        while True:
            insns = list(bb.instructions)
            target = None
            for idx, ins in enumerate(insns):
                si = ins.sync_info
                if si is None:
                    continue
                if len(si.on_wait) > 1:
                    target = (idx, ins)
                    break
            if target is None:
                break
            idx, ins = target
            si = ins.sync_info
            waits = list(si.on_wait)
            if type(ins).__name__ == "InstDrain":
                room = max(0, 1 - len(si.on_update))
            else:
                room = 1
            keep, extra = waits[len(waits) - room :], waits[: len(waits) - room]
            pos = idx
            for i, w in enumerate(extra):
                nd = mybir.InstDrain(
                    name=f"{ins.name}-presync{i}", ins=[], outs=[],
                    bass_is_fusable=False,
                )
                nd.engine = ins.engine
                nd.sync_info = _br.SyncInfo(on_wait=[w], on_update=[])
                nc.register_instruction(nd, overwrite=True)
                bb.instructions.insert(pos, nd)
                pos += 1
            si.on_wait = keep
            ins.sync_info = si
    return nc


_NC_CACHE = {}


def _get_nc():
    if "nc" not in _NC_CACHE:
        _NC_CACHE["nc"] = _build()
    return _NC_CACHE["nc"]


def kernel(logits, labels):
    global LAST_RESULTS
    logits = np.ascontiguousarray(np.asarray(logits), dtype=np.float32)
    labels_i = np.asarray(labels).astype(np.int64)
    assert logits.shape == (N, C), logits.shape

    # host-side gather of the label logit (1% of input bytes; the heavy
    # softmax/max/binning all happen on device)
    gvals = logits[np.arange(N), labels_i].astype(np.float32)
    eye = np.eye(P, dtype=np.float32)
    thr_cols = np.zeros(32, dtype=np.float32)
    for b in range(15):
        thr_cols[b] = -THR[b]
        thr_cols[15 + b] = -np.float32(np.float64(2.0) + np.float64(THR[b]))
    thr_arr = np.broadcast_to(thr_cols, (P, 32)).copy()

    in_maps = []
    for c in range(NCORES):
        sl = slice(c * ROWS, (c + 1) * ROWS)
        in_maps.append(
            {
                "x": logits[sl].reshape(P, SPP * C),
                "g": gvals[sl].reshape(P, SPP),
                "eye": eye,
                "thr": thr_arr,
            }
        )

    trace = bool(int(os.environ.get("ECE_TRACE", "0")))
    res = run_bass_kernel_spmd(
        _get_nc(), in_maps, core_ids=list(range(NCORES)), trace=trace
    )
    LAST_RESULTS = res

    r = np.zeros(NCHUNK * NBINS, np.float64)
    nn_ = np.zeros(NCHUNK * NBINS, np.float64)
    a = np.zeros(NCHUNK * NBINS, np.float64)
    for out in res.results:
        r += out["r"].astype(np.float64).sum(axis=0)
        nn_ += out["nn"].astype(np.float64).sum(axis=0)
        a += out["a"].astype(np.float64).sum(axis=0)
    r = r.reshape(NCHUNK, NBINS).sum(axis=0)
    nn_ = nn_.reshape(NCHUNK, NBINS).sum(axis=0)
    a = a.reshape(NCHUNK, NBINS).sum(axis=0)

    thr64 = np.array([np.float64(t) for t in THR])
    T = r[15]
    n1 = a[15]
    n0 = N - n1
    S = T - r[:15] - thr64 * (N - nn_[:15])
    cumconf = np.concatenate([S, [T]])
    conf_sum = np.diff(cumconf, prepend=0.0)
    K = (N - a[:15]) / 2.0
    cumacc = np.concatenate([K - n0, [n1]])
    acc_sum = np.diff(cumacc, prepend=0.0)
    ece = np.abs(conf_sum - acc_sum).sum() / N
    return np.array([ece], dtype=np.float32)
